# revision 3
# baseline (speedup 1.0000x reference)
"""Trainium2 Bass kernel for nn_BoneLinear: out = x @ W^T + pooled(x) @ disha.

Identity used: pooled(x) @ disha == x @ A where A[j, o] = disha[j % 64, o]
(vertical tiling of disha).  So the whole module is one matmul:
    out = x @ (W^T + tile(disha, 16))
The effective weight W_eff^T is built on-device once per core (PE transpose of
W + DVE add of the tiled disha), then a single dense f32r matmul streams over
the core's batch shard.

Sharding: pure data-parallel over batch (B=8 -> one batch element per core).
Each core reads its x shard [4096, 1024], full weight and disha, and writes
its output shard [4096, 1024].  No collectives.
"""

import sys
import os

for _p in ("/opt/trn_rl_repo", "/root/.axon_site/_ro/trn_rl_repo"):
    if os.path.isdir(_p) and _p not in sys.path:
        sys.path.insert(0, _p)

import numpy as np

import concourse.bass as bass
import concourse.mybir as mybir
import concourse.tile as tile
from concourse import bacc
from concourse.bass_utils import run_bass_kernel_spmd
from concourse.masks import make_identity

# Problem shapes (hardcoded per contract)
B, S, D_IN, D_OUT, R = 8, 4096, 1024, 1024, 64
N_CORES = 8
P = 128
KO = D_IN // P          # 8 contraction chunks of 128
OC = D_OUT // P         # 8 output chunks of 128 (for W transpose)
MT = S // P             # 32 token tiles per core
NF = 512                # matmul moving free dim (one PSUM bank of fp32)
NT = D_OUT // NF        # 2 n-tiles

F32 = mybir.dt.float32
F32R = mybir.dt.float32r


def build_bass():
    nc = bacc.Bacc("TRN2", target_bir_lowering=False, debug=False, num_devices=1)
    x_ap = nc.dram_tensor("x", [S, D_IN], F32, kind="ExternalInput").ap()
    w_ap = nc.dram_tensor("w", [D_OUT, D_IN], F32, kind="ExternalInput").ap()
    d_ap = nc.dram_tensor("disha", [R, D_OUT], F32, kind="ExternalInput").ap()
    out_ap = nc.dram_tensor("out", [S, D_OUT], F32, kind="ExternalOutput").ap()

    with tile.TileContext(nc) as tc:
        with (
            tc.tile_pool(name="const", bufs=1) as const,
            tc.tile_pool(name="wp", bufs=1) as wpool,
            tc.tile_pool(name="xp", bufs=3) as xpool,
            tc.tile_pool(name="xtp", bufs=3) as xtpool,
            tc.tile_pool(name="op", bufs=3) as opool,
            tc.tile_pool(name="pstp", bufs=3, space="PSUM") as psum_tp,
            tc.tile_pool(name="psacc", bufs=2, space="PSUM") as psum_acc,
        ):
            ident = const.tile([P, P], F32)
            make_identity(nc, ident)

            # disha tiled twice on partitions: disha2[p, :] = disha[p % 64, :]
            disha2 = const.tile([P, D_OUT], F32)
            nc.sync.dma_start(disha2[0:R, :], d_ap[:, :])
            nc.sync.dma_start(disha2[R : 2 * R, :], d_ap[:, :])

            # Build W_eff^T[p + 128*kc, oc*128 + q] = W[q(within oc), p(of kc)] + disha2[p]
            w_eff = wpool.tile([P, KO, D_OUT], F32R)
            with tc.tile_pool(name="wnat", bufs=1) as wnat_pool:
                w_nat = wnat_pool.tile([P, OC, D_IN], F32)
                nc.sync.dma_start(
                    w_nat[:], w_ap.rearrange("(oc p) d -> p oc d", p=P)
                )
                for oc in range(OC):
                    for kc in range(KO):
                        pst = psum_tp.tile([P, P], F32, tag="tp")
                        nc.tensor.transpose(
                            pst[:], w_nat[:, oc, kc * P : (kc + 1) * P], ident[:]
                        )
                        nc.vector.tensor_add(
                            w_eff[:, kc, oc * P : (oc + 1) * P],
                            pst[:],
                            disha2[:, oc * P : (oc + 1) * P],
                        )

            # Main loop over token tiles
            for m in range(MT):
                x_t = xpool.tile([P, D_IN], F32)
                nc.sync.dma_start(x_t[:], x_ap[m * P : (m + 1) * P, :])

                xT = xtpool.tile([P, KO, P], F32R)
                for kc in range(KO):
                    pst = psum_tp.tile([P, P], F32, tag="tp")
                    nc.tensor.transpose(
                        pst[:], x_t[:, kc * P : (kc + 1) * P], ident[:]
                    )
                    nc.vector.tensor_copy(xT[:, kc], pst[:])

                o_sb = opool.tile([P, D_OUT], F32)
                pss = [
                    psum_acc.tile([P, NF], F32, tag=f"acc{n}", name=f"acc_{m}_{n}")
                    for n in range(NT)
                ]
                for kc in range(KO):
                    for n in range(NT):
                        nc.tensor.matmul(
                            pss[n][:],
                            xT[:, kc],
                            w_eff[:, kc, n * NF : (n + 1) * NF],
                            start=(kc == 0),
                            stop=(kc == KO - 1),
                        )
                for n in range(NT):
                    nc.any.tensor_copy(o_sb[:, n * NF : (n + 1) * NF], pss[n][:])
                nc.sync.dma_start(out_ap[m * P : (m + 1) * P, :], o_sb[:])

    nc.compile()
    return nc


def kernel(x: np.ndarray, weight: np.ndarray, disha: np.ndarray) -> np.ndarray:
    assert x.shape == (B, S, D_IN) and weight.shape == (D_OUT, D_IN)
    assert disha.shape == (R, D_OUT)
    nc = build_bass()
    x = np.ascontiguousarray(x, dtype=np.float32)
    weight = np.ascontiguousarray(weight, dtype=np.float32)
    disha = np.ascontiguousarray(disha, dtype=np.float32)
    in_maps = [
        {"x": x[c], "w": weight, "disha": disha} for c in range(N_CORES)
    ]
    res = run_bass_kernel_spmd(nc, in_maps, core_ids=list(range(N_CORES)))
    out = np.stack([res.results[c]["out"] for c in range(N_CORES)], axis=0)
    return out


if __name__ == "__main__":
    rng = np.random.default_rng(0)
    x = rng.standard_normal((B, S, D_IN), dtype=np.float32)
    w = (rng.standard_normal((D_OUT, D_IN), dtype=np.float32) / 32.0).astype(
        np.float32
    )
    d = (rng.standard_normal((R, D_OUT), dtype=np.float32) * 0.01).astype(np.float32)
    out = kernel(x=x, weight=w, disha=d)
    print(out.shape, out.dtype)
